# revision 13
# baseline (speedup 1.0000x reference)
"""Trainium2 Bass kernel for EnhancedGraphConvEncoder (8 NeuronCores, SPMD).

Strategy (node-sharded data parallel):
  - NC cores each own N/NC contiguous nodes (= G/NC graphs of 256 nodes).
  - GCN aggregation is linear: aggregate in feature space FIRST (segment-sum
    over edges via one-hot matmuls on the tensor engine), apply the layer
    weight AFTER the reduction.
  - Gather sources are fp16 row tables in DRAM: stage 1 gathers x*norm
    (host-prepared, replicated); residual stages gather feat*norm tables
    built on device and exchanged with AllGather.
  - Edge lists are sorted by dst and padded per 128-dst tile on the host;
    dst-local one-hot matrices are built on the DVE via is_equal.
  - Per-graph MHA runs fully on-core; softmax uses exp without max
    subtraction (scores are tiny for this model family) with the
    denominator extracted via a ones-column in the value matrix.
  - Final BatchNorm (over the 128-graph batch) is applied on host.
  - ALL constants are packed into ONE flat fp16-typed input tensor, carved
    into fp16/int16/int8/f32 sections on device via bitcast views: the
    PJRT/axon dispatch path costs ~0.7-1.5 ms per input tensor per call
    plus ~0.8 ms per MB of input bytes, so one small input wins. The x*norm
    gather table is uploaded as a 2 MB per-core shard and assembled on
    device with an AllGather instead of a 16 MB replicated upload.
  - fp16 is used for gather tables and stored weights (cast to f32 at load
    time); activations that feed mean/max pooling stay f32 — quantizing them
    amplifies through the final BatchNorm's small per-feature batch std.
"""

import numpy as np

import concourse.bacc as bacc
import concourse.tile as tile
import concourse.bass as bass
import concourse.mybir as mybir
import concourse.bass_utils as bass_utils

dt = mybir.dt
AF = mybir.ActivationFunctionType
ALU = mybir.AluOpType

P = 128
N_CORES = 8
G, S = 128, 256
N = G * S              # 32768
H = 256
DR = 256
NH, DH = 8, 32
L = 2
EPS = 1e-5
SHARD = N // N_CORES   # 4096
TILES = SHARD // P     # 32
GPG = G // N_CORES     # 16 graphs per core
ISQ = 1.0 / np.sqrt(DH)

# Chunked-AllGather row permutation: tables are stored so that AllGather
# chunk k (local rows [k*CHR, (k+1)*CHR) of every core) writes one
# contiguous [N_CORES*CHR, H] block. Node n = c*SHARD + r lives at table
# row PI[n] = (r//CHR)*N_CORES*CHR + c*CHR + (r%CHR).
NCHK = 4               # AllGather chunks per table
CHR = SHARD // NCHK    # 1024 local rows per chunk
_n_ = np.arange(N)
PI = ((_n_ % SHARD) // CHR) * (N_CORES * CHR) + (_n_ // SHARD) * CHR + (_n_ % CHR)

bf16 = dt.bfloat16
f16 = dt.float16
f32 = dt.float32
i16 = dt.int16


# ---------------------------------------------------------------- host prep

def _prep_edges(edge_index):
    """Sort edges (plus self-loops) by dst, pad per 128-dst tile to a
    uniform chunk count, and build per-core gather-index / dst-local /
    norm arrays."""
    src = np.asarray(edge_index[0], np.int64)
    dst = np.asarray(edge_index[1], np.int64)
    E = src.shape[0]
    deg = np.bincount(dst, minlength=N).astype(np.float32) + 1.0
    norm = deg ** -0.5

    sall = np.concatenate([src, np.arange(N, dtype=np.int64)])
    dall = np.concatenate([dst, np.arange(N, dtype=np.int64)])
    order = np.argsort(dall, kind="stable")
    ssrc = sall[order]
    sdst = dall[order]

    n_tiles_g = N // P  # 256 global dst tiles
    tile_of = sdst // P
    cnt = np.bincount(tile_of, minlength=n_tiles_g)
    nchunk = int(np.ceil(cnt.max() / P))
    slots_per_tile = nchunk * P

    # slot arrays [n_tiles_g, slots]: src id (pad=0) and dst-local (pad=-1)
    src_slots = np.zeros((n_tiles_g, slots_per_tile), np.int64)
    dl_slots = np.full((n_tiles_g, slots_per_tile), -1.0, np.float32)
    starts = np.zeros(n_tiles_g + 1, np.int64)
    np.cumsum(cnt, out=starts[1:])
    pos_in_tile = np.arange(len(sdst)) - starts[tile_of]
    src_slots[tile_of, pos_in_tile] = ssrc
    dl_slots[tile_of, pos_in_tile] = (sdst - tile_of * P).astype(np.float32)

    # gather tables live in PI-permuted row order (see PI above)
    src_slots = PI[src_slots]
    # sort slots by (permuted) src within each tile for HBM page locality;
    # the one-hot encodes dst per slot, so any slot order is valid as long
    # as src/dstloc stay aligned
    so = np.argsort(src_slots, axis=1, kind="stable")
    src_slots = np.take_along_axis(src_slots, so, axis=1)
    dl_slots = np.take_along_axis(dl_slots, so, axis=1)

    idx_maps, dl_maps, norm_maps = [], [], []
    for c in range(N_CORES):
        t0 = c * TILES
        # gather idx layout: per tile, element j at [j%16, j//16]; stored as
        # 16 rows only — replicated x8 across partitions at load time
        s_c = src_slots[t0 : t0 + TILES]          # [TILES, slots]
        idx16 = s_c.reshape(TILES, slots_per_tile // 16, 16)
        idx16 = idx16.transpose(2, 0, 1).reshape(16, TILES * (slots_per_tile // 16))
        idx_maps.append(np.ascontiguousarray(idx16.astype(np.int16)))

        d_c = dl_slots[t0 : t0 + TILES]            # [TILES, slots]
        dlm = d_c.reshape(TILES, nchunk, P).transpose(2, 0, 1).reshape(P, TILES * nchunk)
        # note: element j of chunk k at [j%128 -> partition, chunk col]
        dl_maps.append(np.ascontiguousarray(dlm.astype(np.float32)))

        nc_col = norm[c * SHARD : (c + 1) * SHARD].reshape(TILES, P).T
        norm_maps.append(np.ascontiguousarray(nc_col.astype(np.float32)))

    return nchunk, norm, idx_maps, dl_maps, norm_maps


def _bcast(v):
    return np.ascontiguousarray(np.tile(np.asarray(v, np.float32)[None, :], (P, 1)))


def _col(v):
    """[H] vector -> [P, H//P] column layout (f at [f%128, f//128])."""
    v = np.asarray(v, np.float32)
    return np.ascontiguousarray(v.reshape(-1, P).T)


# f32 constant-pack order: (name, shape) — dstloc first (dynamic size).
def _pack_f32(parts):
    """parts: list of (name, array). Returns flat f32 array + offsets."""
    offs, bufs, o = {}, [], 0
    for name, arr in parts:
        a = np.ascontiguousarray(np.asarray(arr, np.float32)).ravel()
        offs[name] = (o, tuple(np.asarray(arr).shape))
        bufs.append(a)
        o += a.size
    return np.concatenate(bufs), offs


def prep_inputs(inputs):
    x = np.asarray(inputs["x"], np.float32)
    nchunk, norm, idx_maps, dl_maps, norm_maps = _prep_edges(
        np.asarray(inputs["edge_index"])
    )

    xs16 = (x * norm[:, None]).astype(np.float16)   # [N, H]
    xs16 = xs16[np.argsort(PI)]                     # PI-permuted row order

    W_local = np.asarray(inputs["W_local"], np.float32)
    W_global = np.asarray(inputs["W_global"], np.float32)
    Wcat = np.concatenate([W_local, W_global], axis=1)          # [DR, H]
    bcat = np.concatenate(
        [np.asarray(inputs["b_local"], np.float32), np.asarray(inputs["b_global"], np.float32)]
    )

    ipw = np.asarray(inputs["in_proj_w"], np.float32)            # [3H, H]
    ipb = np.asarray(inputs["in_proj_b"], np.float32)            # [3H]
    opw = np.asarray(inputs["out_proj_w"], np.float32)           # [H, H]
    opb = np.asarray(inputs["out_proj_b"], np.float32)
    bout_eff = opb + opw @ ipb[2 * H :]                          # fold v bias

    iota = np.tile(np.arange(P, dtype=np.float32)[None, :], (P, 1))
    ident = np.eye(P, dtype=np.float32)
    onesrow = np.ones((1, P), np.float32)
    # band indicator rows: e4[j, m] = 1 if m in [j*32, (j+1)*32)
    e4 = (np.arange(P)[None, :] // DH == np.arange(4)[:, None]).astype(np.float32)

    resW = np.asarray(inputs["res_W"], np.float32)               # [2, H, H]
    resb = np.asarray(inputs["res_b"], np.float32)
    lng = np.asarray(inputs["res_ln_g"], np.float32)
    lnb = np.asarray(inputs["res_ln_b"], np.float32)

    # weights live in fp16 (halves bytes, 4x faster PE than fp32r)
    f16_shared = [
        ("iota", iota),
        ("gatew_col", _col(np.asarray(inputs["gate_w"], np.float32)[:, 0])),
        ("Wcat", Wcat),
        ("iprojT", ipw.T),
        ("WoutT", opw.T),
        ("resW0", resW[0]),
        ("resW1", resW[1]),
        ("combWT", np.asarray(inputs["comb_W"], np.float32).T),
    ]
    # free-dim bias/scale vectors: stored as [1, H] rows, broadcast on device
    f32_shared = [
        ("ident", ident),
        ("onesrow", onesrow),
        ("e4", e4),
        ("bcat_r", bcat.reshape(1, H)),
        ("qb_col", _col(ipb[:H])),
        ("kb_col", _col(ipb[H : 2 * H])),
        ("bout_r", bout_eff.reshape(1, H)),
        ("resb0_r", resb[0].reshape(1, H)),
        ("resb1_r", resb[1].reshape(1, H)),
        ("lng0_r", lng[0].reshape(1, H)),
        ("lng1_r", lng[1].reshape(1, H)),
        ("lnb0_r", lnb[0].reshape(1, H)),
        ("lnb1_r", lnb[1].reshape(1, H)),
        ("gateb", np.asarray(inputs["gate_b"], np.float32).reshape(1, 1)),
        ("epsc", np.full((P, 1), EPS, np.float32)),
        ("combb_col", _col(np.asarray(inputs["comb_b"], np.float32))),
    ]

    def _pack_f16(parts):
        offs, bufs, o = {}, [], 0
        for name, arr in parts:
            a = np.ascontiguousarray(np.asarray(arr, np.float16)).ravel()
            offs[name] = (o, tuple(np.asarray(arr).shape))
            bufs.append(a)
            o += a.size
        return np.concatenate(bufs), offs

    # Single mega input (fp16-typed flat buffer), bitcast-carved on device:
    #   [xs_full | fp16 consts (incl dstloc) | idx(i16) | f32 consts]
    # xs is the FULL replicated x*norm table: gathers read it directly from
    # the input DRAM buffer, so stage 1 needs no AllGather at all.
    xs_flat = xs16.ravel()
    in_maps, offs16, offs32 = [], None, None
    for c in range(N_CORES):
        c16, offs16 = _pack_f16(f16_shared)
        p32 = [("normc", norm_maps[c])] + f32_shared
        cf32, offs32 = _pack_f32(p32)
        idxr = idx_maps[c].ravel()
        dl8 = dl_maps[c].astype(np.int8).ravel()             # values in [-1, 127]
        if c16.size % 2:
            c16 = np.concatenate([c16, np.zeros(1, np.float16)])
        if idxr.size % 2:
            idxr = np.concatenate([idxr, np.zeros(1, np.int16)])
        if dl8.size % 2:
            dl8 = np.concatenate([dl8, np.zeros(1, np.int8)])
        mega = np.concatenate([
            xs_flat,                                         # N*H fp16
            c16,                                             # fp16 consts
            idxr.view(np.float16),                           # idx as raw bits
            dl8.view(np.float16),                            # dstloc int8 bits
            cf32.view(np.float16),                           # f32 as raw bits
        ])
        in_maps.append({"mega": mega})
    c16_len = ((sum(int(np.prod(np.asarray(a).shape)) for _, a in
                    f16_shared) + 1) // 2) * 2
    idx_len = ((idx_maps[0].size + 1) // 2) * 2
    dl_len = ((dl_maps[0].size + 1) // 2)     # fp16 elems covering int8 bytes
    lay = {
        "xs": 0,
        "f16": N * H,
        "idx": N * H + c16_len,
        "dl8": N * H + c16_len + idx_len,
        "f32": N * H + c16_len + idx_len + dl_len,
        "idx_cols": idx_maps[0].shape[1],
        "dl_cols": dl_maps[0].shape[1],
        "total": in_maps[0]["mega"].size,
    }
    return nchunk, (offs16, offs32, lay), in_maps


# ---------------------------------------------------------------- kernel IR

def build(nchunk, offs_lay, debug=False, upto="full", reps=1):
    offs16, offs32, lay = offs_lay
    nc = bacc.Bacc("TRN2", target_bir_lowering=False, debug=False,
                   num_devices=N_CORES)

    mega = nc.dram_tensor("mega", [lay["total"]], f16, kind="ExternalInput").ap()
    xs_full = mega[lay["xs"] : lay["xs"] + N * H].rearrange(
        "(n c) -> n c", c=H)                                     # [N, H] fp16
    F16 = lay["f16"]
    F32 = lay["f32"]

    def f16view(o, n):
        return mega[F16 + o : F16 + o + n]

    def f32view(o, n):
        return mega[F32 + 2 * o : F32 + 2 * (o + n)].bitcast(f32)

    outT = nc.dram_tensor("outT", [H, GPG], f32, kind="ExternalOutput").ap()
    dbg = {}
    if debug:
        for nm in ("feat1T", "feat2", "feat3", "feat4"):
            shp = [P, 2, SHARD] if nm == "feat1T" else [P, TILES, H]
            dbg[nm] = nc.dram_tensor("dbg_" + nm, shp, f32, kind="ExternalOutput").ap()

    with tile.TileContext(nc) as tc:
        with (
            tc.tile_pool(name="const", bufs=1) as cp,
            tc.tile_pool(name="feat", bufs=1) as fp,
            tc.tile_pool(name="work", bufs=2) as wp,
            tc.tile_pool(name="gath", bufs=2) as gp,
            tc.tile_pool(name="psum", bufs=2, space="PSUM") as pp,
            tc.tile_pool(name="psmm", bufs=2, space="PSUM") as pm,
            tc.tile_pool(name="dram", bufs=1, space="DRAM") as dp,
        ):
            # ---------- load constants into SBUF
            _ldn = [0]

            def ldv(view, shape, dtype):
                _ldn[0] += 1
                t = cp.tile(shape, dtype, tag=f"c{_ldn[0]}")
                nc.sync.dma_start(t[:], view)
                return t

            def ldc(name):
                o, shp = offs32[name]
                n = int(np.prod(shp))
                if len(shp) == 1:
                    view = f32view(o, n).rearrange("(a b) -> a b", a=1)
                    shape = [1, shp[0]]
                elif len(shp) == 2:
                    view = f32view(o, n).rearrange("(p x) -> p x", p=shp[0])
                    shape = list(shp)
                return ldv(view, shape, f32)

            def ldc16(name, dtype=f16):
                o, shp = offs16[name]
                n = int(np.prod(shp))
                view = f16view(o, n).rearrange("(p x) -> p x", p=shp[0])
                _ldn[0] += 1
                t = cp.tile(list(shp), dtype, tag=f"c{_ldn[0]}")
                nc.gpsimd.dma_start(t[:], view)  # casts fp16 -> dtype
                return t

            def ldw(name, X):
                """fp16-stored [H, X] weight -> [P, 2, X] f32 (cast in DMA)."""
                o, shp = offs16[name]
                n = int(np.prod(shp))
                view = f16view(o, n).rearrange("(k p x) -> p k x", p=P, x=X)
                _ldn[0] += 1
                t = cp.tile([P, 2, X], f32, tag=f"c{_ldn[0]}")
                nc.gpsimd.dma_start(t[:], view)
                return t

            # idx: stored as 16 rows; replicate x8 across partition groups
            XC = lay["idx_cols"]
            idx_sb = cp.tile([P, XC], i16, tag="idx")
            idx_view = mega[lay["idx"] : lay["idx"] + 16 * XC].bitcast(i16) \
                .rearrange("(p x) -> p x", p=16)
            for k in range(8):
                nc.sync.dma_start(idx_sb[16 * k : 16 * (k + 1), :], idx_view)
            DC = lay["dl_cols"]
            dl8_sb = cp.tile([P, DC], dt.int8, tag="dl8")
            nc.sync.dma_start(
                dl8_sb[:],
                mega[lay["dl8"] : lay["dl8"] + (DC * P + 1) // 2]
                .bitcast(dt.int8)[0 : P * DC].rearrange("(p x) -> p x", p=P))
            dl_sb = cp.tile([P, DC], f32, tag="dl")
            nc.vector.tensor_copy(dl_sb[:], dl8_sb[:])
            nrm_sb = ldc("normc")
            iota_sb = ldc16("iota", f16)
            id_sb = ldc("ident")
            ones_sb = ldc("onesrow")
            e4o, _ = offs32["e4"]
            e4_sb = [ldv(f32view(e4o + j * P, P)
                         .rearrange("(a b) -> a b", a=1), [1, P], f32)
                     for j in range(4)]
            Wcat_sb = ldw("Wcat", H)
            iprojT_sb = ldw("iprojT", 3 * H)
            WoutT_sb = ldw("WoutT", H)
            resW_sb = [ldw("resW0", H), ldw("resW1", H)]
            cwo, _ = offs16["combWT"]
            combWT_sb = cp.tile([P, 6, H], f32, tag="combWT")
            nc.gpsimd.dma_start(combWT_sb[:],
                                f16view(cwo, 3 * H * H)
                                .rearrange("(k p x) -> p k x", p=P, x=H))
            qb_sb = ldc("qb_col")
            kb_sb = ldc("kb_col")
            gw_sb = ldc16("gatew_col", f32)
            gb_sb = ldc("gateb")
            eps_sb = ldc("epsc")
            combb_sb = ldc("combb_col")

            # broadcast [1, H] bias/scale rows to [P, H] via ones-column matmul
            def bcast_row(name):
                o, shp = offs32[name]
                row = ldv(f32view(o, H).rearrange("(a b) -> a b", a=1), [1, H], f32)
                pb = pp.tile([P, H], f32, tag="aux")
                nc.tensor.matmul(pb[:], ones_sb[:], row[:], start=True, stop=True)
                t = cp.tile([P, H], f32, tag=f"bc_{name}")
                nc.scalar.copy(t[:], pb[:])
                return t

            bcat_sb = bcast_row("bcat_r")
            bout_sb = bcast_row("bout_r")
            resb_sb = [bcast_row("resb0_r"), bcast_row("resb1_r")]
            lng_sb = [bcast_row("lng0_r"), bcast_row("lng1_r")]
            lnb_sb = [bcast_row("lnb0_r"), bcast_row("lnb1_r")]

            # ---------- persistent feature buffers
            featT = fp.tile([P, 2, SHARD], f32, tag="featT")   # Lf (f-major)
            feat = fp.tile([P, TILES, H], f32, tag="feat")     # Ln (n-major)

            # ---------- helpers
            def gather2(table, t):
                """fp16 gather for dst tile t -> [P, nchunk, H]."""
                gt = gp.tile([P, nchunk, H], f16, tag="gth")
                nc.gpsimd.dma_gather(
                    out_ap=gt[:],
                    in_ap=table,
                    idxs_ap=idx_sb[:, t * nchunk * 8 : (t + 1) * nchunk * 8],
                    num_idxs=nchunk * P,
                    num_idxs_reg=nchunk * P,
                    elem_size=H,
                    elem_step=H,
                    single_packet=False,
                )
                return gt

            def segsum(gt, t):
                """one-hot segment sum for dst tile t from gathered rows."""
                acc = pm.tile([P, H], f32, tag="mm")
                for k in range(nchunk):
                    oh = wp.tile([P, P], f16, tag="onehot")
                    nc.vector.tensor_scalar(
                        out=oh[:], in0=iota_sb[:],
                        scalar1=dl_sb[:, t * nchunk + k : t * nchunk + k + 1],
                        scalar2=None, op0=ALU.is_equal,
                    )
                    nc.tensor.matmul(acc[:], oh[:], gt[:, k, :],
                                     start=(k == 0), stop=(k == nchunk - 1))
                return acc

            def norm_transpose(acc, t):
                """psum agg -> norm-scaled -> transposed [P,2,P] (fin-major)"""
                aggs = wp.tile([P, H], f32, tag="aggs")
                nc.scalar.activation(aggs[:], acc[:], AF.Copy,
                                     scale=nrm_sb[:, t : t + 1])
                aggT = wp.tile([P, 2, P], f32, tag="aggT")
                for fc in range(2):
                    tp = pp.tile([P, P], f32, tag="aux")
                    nc.tensor.transpose(tp[:], aggs[:, fc * P : (fc + 1) * P], id_sb[:])
                    nc.scalar.copy(aggT[:, fc, :], tp[:])
                return aggT

            def wmat(aggT, W_sb):
                facc = pm.tile([P, H], f32, tag="mm")
                for fc in range(2):
                    nc.tensor.matmul(facc[:], aggT[:, fc, :], W_sb[:, fc, :],
                                     start=(fc == 0), stop=(fc == 1))
                return facc

            def to_f16_table(src_f32, t_i, dst_dram):
                """feat row block [P, H] f32 * norm -> fp16 -> DRAM table rows."""
                sc = wp.tile([P, H], f32, tag="sc")
                nc.scalar.activation(sc[:], src_f32, AF.Copy,
                                     scale=nrm_sb[:, t_i : t_i + 1])
                th = wp.tile([P, H], f16, tag="th")
                nc.vector.tensor_copy(th[:], sc[:])
                nc.sync.dma_start(dst_dram[t_i * P : (t_i + 1) * P, :], th[:])

            for rep in range(reps):
                # fresh shared tables per rep (Shared tensors are single-writer,
                # so each AllGather chunk gets its own Shared tensor and a
                # local DMA coalesces it into the flat gather table)
                t2loc = dp.tile([SHARD, H], f16, tag="t2loc")
                t2chunk = [dp.tile([N_CORES * CHR, H], f16, addr_space="Shared",
                                   tag=f"t2chunk{kc}") for kc in range(NCHK)]
                table2 = dp.tile([N, H], f16, tag="table2")
                t3loc = dp.tile([SHARD, H], f16, tag="t3loc")
                t3chunk = [dp.tile([N_CORES * CHR, H], f16, addr_space="Shared",
                                   tag=f"t3chunk{kc}") for kc in range(NCHK)]
                table3 = dp.tile([N, H], f16, tag="table3")
                # ================= Stage 1: GCN on x' -> feat1T (Lf)
                # (x*norm table is the replicated input itself — no AllGather)
                for t in range(TILES):
                    gt = gather2(xs_full, t)
                    acc = segsum(gt, t)
                    aggT = norm_transpose(acc, t)
                    facc = wmat(aggT, Wcat_sb)
                    ftmp = wp.tile([P, H], f32, tag="ftmp")
                    nc.vector.tensor_tensor(out=ftmp[:], in0=facc[:], in1=bcat_sb[:],
                                            op=ALU.add)
                    frelu = wp.tile([P, H], f32, tag="frelu")
                    nc.scalar.activation(frelu[:], ftmp[:], AF.Relu)
                    for fc in range(2):
                        tp = pp.tile([P, P], f32, tag="aux")
                        nc.tensor.transpose(tp[:], frelu[:, fc * P : (fc + 1) * P], id_sb[:])
                        nc.scalar.copy(featT[:, fc, t * P : (t + 1) * P], tp[:])

                if debug:
                    nc.sync.dma_start(dbg["feat1T"][:], featT[:])

                # ================= Stage 2: per-graph MHA -> feat (Ln) + table2
                run_s2 = upto in ("s2", "s3", "full")
                run_s3 = upto in ("s3", "full")
                run_full = upto == "full"
                # masked k: kTm[:, h, :] holds head h's kT rows (at their natural
                # partition band) and zeros elsewhere, so K=128 matmuls stay
                # base-partition-0 while contracting only head h.
                kTm = fp.tile([P, NH, S], f32, tag="kTm")
                nc.vector.memset(kTm[:], 0.0)
                for g in range(GPG if run_s2 else 0):
                    cols = slice(g * S, (g + 1) * S)
                    # qT [P,2,S]: dtile-major; includes bias and 1/sqrt(dh)
                    qT = wp.tile([P, 2, S], f32, tag="qT")
                    for dtile in range(2):
                        pq = pm.tile([P, S], f32, tag="mm")
                        off = dtile * P
                        for fc in range(2):
                            nc.tensor.matmul(
                                pq[:], iprojT_sb[:, fc, off : off + P],
                                featT[:, fc, cols], start=(fc == 0), stop=(fc == 1))
                        nc.scalar.activation(qT[:, dtile, :], pq[:], AF.Identity,
                                             bias=qb_sb[:, dtile : dtile + 1],
                                             scale=ISQ)
                        pk = pm.tile([P, S], f32, tag="mm")
                        koff = H + dtile * P
                        for fc in range(2):
                            nc.tensor.matmul(
                                pk[:], iprojT_sb[:, fc, koff : koff + P],
                                featT[:, fc, cols], start=(fc == 0), stop=(fc == 1))
                        for j2 in range(4):
                            r0 = j2 * DH
                            h = dtile * 4 + j2
                            nc.scalar.activation(
                                kTm[r0 : r0 + DH, h, :], pk[r0 : r0 + DH, :],
                                AF.Identity, bias=kb_sb[r0 : r0 + DH, dtile : dtile + 1])
                    # v_ext [P, 2, NH*33] (m on partitions; col 32 of each head = 1)
                    v_ext = wp.tile([P, 2, NH * 33], f32, tag="vext")
                    for mt in range(2):
                        pv = pm.tile([P, H], f32, tag="mm")
                        msl = slice(g * S + mt * P, g * S + (mt + 1) * P)
                        for fc in range(2):
                            nc.tensor.matmul(pv[:], featT[:, fc, msl],
                                             iprojT_sb[:, fc, 2 * H : 3 * H],
                                             start=(fc == 0), stop=(fc == 1))
                        vv = v_ext[:, mt, :].rearrange("p (h e) -> p h e", h=NH)
                        nc.scalar.copy(vv[:, :, 0:DH], pv[:].rearrange("p (h d) -> p h d", h=NH))
                        nc.vector.memset(vv[:, :, DH : DH + 1], 1.0)
                    # per-head attention
                    oT = wp.tile([P, 2, S], f32, tag="oT")
                    zinv_all = wp.tile([1, NH * S], f32, tag="zinv")
                    for h in range(NH):
                        r0 = (h % 4) * DH
                        dt_i = h // 4
                        expT = wp.tile([P, 2, S], f32, tag="expT")
                        for mt in range(2):
                            ps = pm.tile([P, S], f32, tag="mm")
                            nc.tensor.matmul(
                                ps[:],
                                kTm[:, h, mt * P : (mt + 1) * P],
                                qT[:, dt_i, :],
                                start=True, stop=True)
                            nc.scalar.activation(expT[:, mt, :], ps[:], AF.Exp)
                        po = pp.tile([DH + 1, S], f32, tag="aux")
                        for mt in range(2):
                            nc.tensor.matmul(po[:], v_ext[:, mt, h * 33 : (h + 1) * 33],
                                             expT[:, mt, :],
                                             start=(mt == 0), stop=(mt == 1))
                        nc.scalar.copy(oT[r0 : r0 + DH, dt_i, :], po[0:DH, :])
                        nc.vector.reciprocal(zinv_all[0:1, h * S : (h + 1) * S],
                                             po[DH : DH + 1, :])
                    # z-expansion: accumulate 4 per-head bands into [P, S] then scale
                    for j in range(2):
                        pz = pm.tile([P, S], f32, tag="mm")
                        for j2 in range(4):
                            h2 = j * 4 + j2
                            nc.tensor.matmul(pz[:], e4_sb[j2][:],
                                             zinv_all[0:1, h2 * S : (h2 + 1) * S],
                                             start=(j2 == 0), stop=(j2 == 3))
                        nc.vector.tensor_tensor(out=oT[:, j, :], in0=oT[:, j, :],
                                                in1=pz[:], op=ALU.mult)
                    # out proj -> feat (Ln) + table2 rows
                    for nt in range(2):
                        pf = pm.tile([P, H], f32, tag="mm")
                        for vc in range(2):
                            nc.tensor.matmul(pf[:], oT[:, vc, nt * P : (nt + 1) * P],
                                             WoutT_sb[:, vc, :],
                                             start=(vc == 0), stop=(vc == 1))
                        t_i = g * 2 + nt
                        nc.vector.tensor_tensor(out=feat[:, t_i, :], in0=pf[:],
                                                in1=bout_sb[:], op=ALU.add)
                        to_f16_table(feat[:, t_i, :], t_i, t2loc)

                if run_s3:
                    # chunked: chunk k departs as soon as graphs [4k, 4k+4)
                    # have written their t2loc rows, overlapping the rest of
                    # the MHA compute (subtile deps make each chunk wait only
                    # on its own rows).
                    for kc in range(NCHK):
                        nc.gpsimd.collective_compute(
                            "AllGather", ALU.bypass,
                            replica_groups=[list(range(N_CORES))],
                            ins=[t2loc[kc * CHR : (kc + 1) * CHR, :].opt()],
                            outs=[t2chunk[kc].opt()],
                        )
                        nc.sync.dma_start(
                            table2[kc * N_CORES * CHR :
                                   (kc + 1) * N_CORES * CHR, :],
                            t2chunk[kc][:])
                if debug and run_s2:
                    nc.sync.dma_start(dbg["feat2"][:], feat[:])

                # ================= Stages 3/4: residual GCN layers
                for layer in range(L if run_s3 else 0):
                    if layer == 1 and not run_full:
                        break
                    tab = table2 if layer == 0 else table3
                    for t in range(TILES):
                        gt = gather2(tab[:], t)
                        acc = segsum(gt, t)
                        aggT = norm_transpose(acc, t)
                        hacc = wmat(aggT, resW_sb[layer])
                        hb = wp.tile([P, H], f32, tag="hb")
                        nc.vector.tensor_tensor(out=hb[:], in0=hacc[:],
                                                in1=resb_sb[layer][:], op=ALU.add)
                        # LayerNorm over free dim
                        musum = wp.tile([P, 1], f32, tag="musum")
                        nc.vector.tensor_reduce(musum[:], hb[:], mybir.AxisListType.X,
                                                ALU.add)
                        mu = wp.tile([P, 1], f32, tag="mu")
                        nc.scalar.activation(mu[:], musum[:], AF.Copy, scale=1.0 / H)
                        hc = wp.tile([P, H], f32, tag="hc")
                        nc.vector.tensor_scalar(out=hc[:], in0=hb[:], scalar1=mu[:],
                                                scalar2=None, op0=ALU.subtract)
                        sq = wp.tile([P, H], f32, tag="lnx")
                        varsum = wp.tile([P, 1], f32, tag="varsum")
                        nc.scalar.activation(sq[:], hc[:], AF.Square,
                                             accum_out=varsum[:])
                        std = wp.tile([P, 1], f32, tag="std")
                        nc.scalar.activation(std[:], varsum[:], AF.Sqrt,
                                             scale=1.0 / H, bias=eps_sb[:, 0:1])
                        rstd = wp.tile([P, 1], f32, tag="rstd")
                        nc.vector.reciprocal(rstd[:], std[:])
                        t1 = wp.tile([P, H], f32, tag="lnx")
                        nc.vector.tensor_scalar(out=t1[:], in0=hc[:], scalar1=rstd[:],
                                                scalar2=None, op0=ALU.mult)
                        t2_ = wp.tile([P, H], f32, tag="lnx")
                        nc.vector.tensor_tensor(out=t2_[:], in0=t1[:],
                                                in1=lng_sb[layer][:], op=ALU.mult)
                        t3_ = wp.tile([P, H], f32, tag="lnx")
                        nc.vector.tensor_tensor(out=t3_[:], in0=t2_[:],
                                                in1=lnb_sb[layer][:], op=ALU.add)
                        rl = wp.tile([P, H], f32, tag="lnx")
                        nc.scalar.activation(rl[:], t3_[:], AF.Relu)
                        nc.vector.tensor_tensor(out=feat[:, t, :], in0=feat[:, t, :],
                                                in1=rl[:], op=ALU.add)
                        if layer == 0:
                            to_f16_table(feat[:, t, :], t, t3loc)
                        else:
                            for fc in range(2):
                                tp = pp.tile([P, P], f32, tag="aux")
                                nc.tensor.transpose(tp[:], feat[:, t, fc * P : (fc + 1) * P],
                                                    id_sb[:])
                                nc.scalar.copy(featT[:, fc, t * P : (t + 1) * P], tp[:])
                    if layer == 0:
                        if run_full:
                            for kc in range(NCHK):
                                nc.gpsimd.collective_compute(
                                    "AllGather", ALU.bypass,
                                    replica_groups=[list(range(N_CORES))],
                                    ins=[t3loc[kc * CHR : (kc + 1) * CHR, :].opt()],
                                    outs=[t3chunk[kc].opt()],
                                )
                                nc.sync.dma_start(
                                    table3[kc * N_CORES * CHR :
                                           (kc + 1) * N_CORES * CHR, :],
                                    t3chunk[kc][:])
                        if debug:
                            nc.sync.dma_start(dbg["feat3"][:], feat[:])
                if debug and run_full:
                    nc.sync.dma_start(dbg["feat4"][:], feat[:])

                # ================= Stage 5: pooling per graph
                poolT = fp.tile([P, 2, 3 * GPG], f32, tag="poolT")
                nc.vector.memset(poolT[:], 0.0)
                for g in range(GPG if run_full else 0):
                    cols = slice(g * S, (g + 1) * S)
                    lg = pp.tile([1, S], f32, tag="aux")
                    for fc in range(2):
                        nc.tensor.matmul(lg[:], gw_sb[:, fc : fc + 1],
                                         featT[:, fc, cols],
                                         start=(fc == 0), stop=(fc == 1))
                    gexp = wp.tile([1, S], f32, tag="gexp")
                    nc.scalar.activation(gexp[:], lg[:], AF.Exp, bias=gb_sb[:, 0:1])
                    zsum = wp.tile([1, 1], f32, tag="zsum")
                    nc.vector.tensor_reduce(zsum[:], gexp[:], mybir.AxisListType.X,
                                            ALU.add)
                    pzc = pp.tile([P, 1], f32, tag="aux")
                    nc.tensor.matmul(pzc[:], ones_sb[:], zsum[:], start=True, stop=True)
                    zic = wp.tile([P, 1], f32, tag="zic")
                    nc.vector.reciprocal(zic[:], pzc[:])
                    pge = pp.tile([P, S], f32, tag="aux")
                    nc.tensor.matmul(pge[:], ones_sb[:], gexp[:], start=True, stop=True)
                    for fc in range(2):
                        wgt = wp.tile([P, S], f32, tag="wgt")
                        nc.vector.tensor_tensor(out=wgt[:], in0=featT[:, fc, cols],
                                                in1=pge[:], op=ALU.mult)
                        araw = wp.tile([P, 1], f32, tag="araw")
                        nc.vector.tensor_reduce(araw[:], wgt[:], mybir.AxisListType.X,
                                                ALU.add)
                        nc.vector.tensor_scalar(out=poolT[:, fc, g : g + 1], in0=araw[:],
                                                scalar1=zic[:], scalar2=None,
                                                op0=ALU.mult)
                        mraw = wp.tile([P, 1], f32, tag="mraw")
                        nc.vector.tensor_reduce(mraw[:], featT[:, fc, cols],
                                                mybir.AxisListType.X, ALU.add)
                        nc.vector.tensor_scalar(out=poolT[:, fc, GPG + g : GPG + g + 1],
                                                in0=mraw[:], scalar1=1.0 / S,
                                                scalar2=None, op0=ALU.mult)
                        nc.vector.tensor_reduce(poolT[:, fc, 2 * GPG + g : 2 * GPG + g + 1],
                                                featT[:, fc, cols],
                                                mybir.AxisListType.X, ALU.max)

                # ================= Stage 6: combine
                outsb = fp.tile([P, 2, GPG], f32, tag="outsb")
                for j in range(2):
                    pg = pp.tile([P, GPG], f32, tag="aux")
                    k = 0
                    for b in range(3):
                        for fc in range(2):
                            nc.tensor.matmul(
                                pg[:], combWT_sb[:, b * 2 + fc, j * P : (j + 1) * P],
                                poolT[:, fc, b * GPG : (b + 1) * GPG],
                                start=(k == 0), stop=(k == 5))
                            k += 1
                    nc.scalar.activation(outsb[:, j, :], pg[:], AF.Identity,
                                         bias=combb_sb[:, j : j + 1])
                nc.sync.dma_start(outT.rearrange("(k p) g -> p k g", p=P), outsb[:])

    nc.compile()
    return nc


# ---------------------------------------------------------------- execution

_CACHE = {}


def _get_nc(nchunk, offs, debug=False, upto="full", reps=1):
    key = (nchunk, debug, upto, reps)
    if key not in _CACHE:
        _CACHE[key] = build(nchunk, offs, debug=debug, upto=upto, reps=reps)
    return _CACHE[key]


def _host_bn(outs, inputs):
    g_pre = np.zeros((G, H), np.float32)
    for c in range(N_CORES):
        g_pre[c * GPG : (c + 1) * GPG, :] = outs[c]["outT"].T
    bm = g_pre.mean(axis=0)
    bv = ((g_pre - bm) ** 2).mean(axis=0)
    bn_g = np.asarray(inputs["bn_g"], np.float32)
    bn_b = np.asarray(inputs["bn_b"], np.float32)
    return ((g_pre - bm) / np.sqrt(bv + EPS) * bn_g + bn_b).astype(np.float32)


def kernel(**inputs):
    nchunk, offs, in_maps = prep_inputs(inputs)
    nc = _get_nc(nchunk, offs)
    res = bass_utils.run_bass_kernel_spmd(nc, in_maps,
                                          core_ids=list(range(N_CORES)))
    return _host_bn(res.results, inputs)


def kernel_debug(upto="full", **inputs):
    nchunk, offs, in_maps = prep_inputs(inputs)
    nc = _get_nc(nchunk, offs, debug=True, upto=upto)
    res = bass_utils.run_bass_kernel_spmd(nc, in_maps,
                                          core_ids=list(range(N_CORES)))
    return _host_bn(res.results, inputs), res.results


# ------------------------------------------------------------- timed runner

def make_runner(nc, in_maps):
    """jit once (no donation), device_put inputs once; returns callable that
    executes the NEFF on all 8 cores and blocks."""
    import jax
    import numpy as _np
    from jax.sharding import Mesh, PartitionSpec
    from jax.experimental.shard_map import shard_map
    import concourse.mybir as _mybir
    from concourse import bass2jax as _b2j

    _b2j.install_neuronx_cc_hook()
    partition_name = (nc.partition_id_tensor.name
                      if nc.partition_id_tensor else None)
    in_names, out_names, out_avals, zero_outs = [], [], [], []
    for alloc in nc.m.functions[0].allocations:
        if not isinstance(alloc, _mybir.MemoryLocationSet):
            continue
        name = alloc.memorylocations[0].name
        if alloc.kind == "ExternalInput":
            if name != partition_name:
                in_names.append(name)
        elif alloc.kind == "ExternalOutput":
            shape = tuple(alloc.tensor_shape)
            dtype = _mybir.dt.np(alloc.dtype)
            out_names.append(name)
            out_avals.append(jax.core.ShapedArray(shape, dtype))
            zero_outs.append(_np.zeros(shape, dtype))
    n_params = len(in_names)
    all_in_names = list(in_names) + list(out_names)
    if partition_name is not None:
        all_in_names.append(partition_name)

    def _body(*args):
        operands = list(args)
        if partition_name is not None:
            operands.append(_b2j.partition_id_tensor())
        outs = _b2j._bass_exec_p.bind(
            *operands,
            out_avals=tuple(out_avals),
            in_names=tuple(all_in_names),
            out_names=tuple(out_names),
            lowering_input_output_aliases=(),
            sim_require_finite=True,
            sim_require_nnan=True,
            nc=nc,
        )
        return tuple(outs)

    devices = jax.devices()[:N_CORES]
    mesh = Mesh(_np.asarray(devices), ("core",))
    in_specs = (PartitionSpec("core"),) * (n_params + len(out_names))
    out_specs = (PartitionSpec("core"),) * len(out_names)
    fn = jax.jit(shard_map(_body, mesh=mesh, in_specs=in_specs,
                           out_specs=out_specs, check_rep=False),
                 keep_unused=True)

    concat_in = [
        _np.concatenate([_np.asarray(in_maps[c][nm]) for c in range(N_CORES)], axis=0)
        for nm in in_names
    ] + [
        _np.concatenate([z] * N_CORES, axis=0) for z in zero_outs
    ]
    dev_args = jax.device_put(concat_in)
    for a in dev_args:
        a.block_until_ready()

    def run():
        outs = fn(*dev_args)
        for o in outs:
            o.block_until_ready()
        return outs

    run.launch = lambda: fn(*dev_args)   # async launch (no block)
    return run, out_names


def kernel_timed(inputs, iters=12, reps=1):
    """Returns (output, [per-iter seconds])."""
    import time as _t
    nchunk, offs, in_maps = prep_inputs(inputs)
    nc = _get_nc(nchunk, offs, reps=reps)
    run, out_names = make_runner(nc, in_maps)
    run()  # compile + warm
    times = []
    for _ in range(iters):
        t0 = _t.perf_counter()
        outs = run()
        times.append(_t.perf_counter() - t0)
    g = np.asarray(outs[out_names.index("outT")])  # [8*H, GPG]
    per_core = [{"outT": g[c * H : (c + 1) * H]} for c in range(N_CORES)]
    return _host_bn(per_core, inputs), times

